# revision 1
# baseline (speedup 1.0000x reference)
"""Trainium2 Bass kernel for nn_Decoder_14894946582933.

Pipeline (per timestep s, per core):
    Y[s]  = conv3x3(X[s]) + conv_b          (PE: 9 offsets x 2 cin-chunks x 2 cout-groups)
    Z[s]  = dv_c * head_w @ Y[s]            (PE: 21 output channels, dv_c folded into weights)
    w_s   = alpha * w_{s-1} + Z[s]          (DVE scalar_tensor_tensor; w := dv_c * i  LI state)
    v_s   = beta  * v_{s-1} + w_{s-1}       (DVE; v is the emitted LI membrane voltage)
    out[s] = v_s + head_b                   (ACT bias add)

Sharding: 8 cores = 4 batch x 2 H-halves (halo of 1 row via host padding).
No collectives. Matmuls run in float32r (TF32-class, 1 cycle/row).
"""

import numpy as np

import concourse.mybir as mybir
from concourse import bacc
from concourse.tile import TileContext
from concourse.bass_utils import run_bass_kernel_spmd

# Problem shapes (hardcoded per contract)
TS = 16          # timesteps
B = 4            # batch
C = 256          # channels
H = W = 48       # spatial
OB, OC = 12, 9   # box / class output channels
NO = OB + OC     # 21 head outputs
HH = 24          # H rows per core (H/2)
PH, PW = HH + 2, W + 2   # padded slab: 26 x 50
ROWS = 8         # output rows per spatial chunk
NCH = HH // ROWS          # 3 spatial chunks
NF = ROWS * W             # 384 matmul free size
PIX = HH * W              # 1152 pixels per core

# norse LIParameters
DT_ = 1e-3
DV_C = DT_ * 100.0   # 0.1
DI_C = DT_ * 200.0   # 0.2
ALPHA = 1.0 - DI_C   # 0.8
BETA = 1.0 - DV_C    # 0.9

F32 = mybir.dt.float32
F32R = mybir.dt.float32r

_CACHED = None


def _build():
    nc = bacc.Bacc(None, target_bir_lowering=False)

    xp = nc.dram_tensor("Xp", [TS, 2, 128, PH * PW], F32R, kind="ExternalInput")
    wc = nc.dram_tensor("Wc", [128, 36 * 128], F32R, kind="ExternalInput")
    wh = nc.dram_tensor("Wh", [128, 2 * NO], F32R, kind="ExternalInput")
    cb = nc.dram_tensor("CB", [128, 2], F32, kind="ExternalInput")
    hb = nc.dram_tensor("HB", [NO, 1], F32, kind="ExternalInput")
    o = nc.dram_tensor("O", [TS, NO, PIX], F32, kind="ExternalOutput")

    with TileContext(nc) as tc:
        with (
            tc.tile_pool(name="wpool", bufs=1) as wpool,
            tc.tile_pool(name="xpool", bufs=3) as xpool,
            tc.tile_pool(name="ypool", bufs=4) as ypool,
            tc.tile_pool(name="spool", bufs=2) as spool,
            tc.tile_pool(name="opool", bufs=2) as opool,
            tc.tile_pool(name="cpsum", bufs=4, space="PSUM") as cpsum,
            tc.tile_pool(name="zpsum", bufs=4, space="PSUM") as zpsum,
        ):
            wc_sb = wpool.tile([128, 36 * 128], F32R)
            wh_sb = wpool.tile([128, 2 * NO], F32R)
            cb_sb = wpool.tile([128, 2], F32)
            hb_sb = wpool.tile([NO, 1], F32)
            nc.sync.dma_start(out=wc_sb[:], in_=wc[:])
            nc.sync.dma_start(out=wh_sb[:], in_=wh[:])
            nc.sync.dma_start(out=cb_sb[:], in_=cb[:])
            nc.sync.dma_start(out=hb_sb[:], in_=hb[:])

            # LI state (v, w); start at zero
            v_prev = spool.tile([NO, PIX], F32, tag="v")
            w_prev = spool.tile([NO, PIX], F32, tag="w")
            nc.vector.memset(v_prev[:], 0.0)
            nc.vector.memset(w_prev[:], 0.0)

            for s in range(TS):
                xts = []
                for ci in range(2):
                    xt = xpool.tile([128, PH * PW], F32R, tag=f"x{ci}")
                    nc.sync.dma_start(out=xt[:], in_=xp[s, ci])
                    xts.append(xt.rearrange("p (h w) -> p h w", h=PH))

                yts = []
                for g in range(2):
                    yt = ypool.tile([128, PIX], F32R, tag=f"y{g}")
                    for c in range(NCH):
                        ps = cpsum.tile([128, NF], F32, tag="cp")
                        n_mm = 0
                        for ci in range(2):
                            for ky in range(3):
                                for kx in range(3):
                                    idx = (ci * 9 + ky * 3 + kx) * 2 + g
                                    rhs = xts[ci][
                                        :,
                                        c * ROWS + ky : c * ROWS + ky + ROWS,
                                        kx : kx + W,
                                    ]
                                    nc.tensor.matmul(
                                        ps[:],
                                        wc_sb[:, idx * 128 : (idx + 1) * 128],
                                        rhs,
                                        start=(n_mm == 0),
                                        stop=(n_mm == 17),
                                    )
                                    n_mm += 1
                        # evacuate + conv bias + round to f32r
                        nc.scalar.activation(
                            yt[:, c * NF : (c + 1) * NF],
                            ps[:],
                            mybir.ActivationFunctionType.Identity,
                            bias=cb_sb[:, g : g + 1],
                        )
                    yts.append(yt)

                # head matmuls: Z[s] = (dv_c * head_w) @ Y[s]
                zps = []
                for c in range(NCH):
                    zp = zpsum.tile([NO, NF], F32, tag="zp")
                    for g in range(2):
                        nc.tensor.matmul(
                            zp[:],
                            wh_sb[:, g * NO : (g + 1) * NO],
                            yts[g][:, c * NF : (c + 1) * NF],
                            start=(g == 0),
                            stop=(g == 1),
                        )
                    zps.append(zp)

                # LI scan step (order matters: v uses w_{s-1})
                v_s = spool.tile([NO, PIX], F32, tag="v")
                nc.vector.scalar_tensor_tensor(
                    out=v_s[:],
                    in0=v_prev[:],
                    scalar=BETA,
                    in1=w_prev[:],
                    op0=mybir.AluOpType.mult,
                    op1=mybir.AluOpType.add,
                )
                w_s = spool.tile([NO, PIX], F32, tag="w")
                for c in range(NCH):
                    nc.vector.scalar_tensor_tensor(
                        out=w_s[:, c * NF : (c + 1) * NF],
                        in0=w_prev[:, c * NF : (c + 1) * NF],
                        scalar=ALPHA,
                        in1=zps[c][:],
                        op0=mybir.AluOpType.mult,
                        op1=mybir.AluOpType.add,
                    )
                v_prev, w_prev = v_s, w_s

                # output = v_s + head bias
                ot = opool.tile([NO, PIX], F32, tag="o")
                nc.scalar.activation(
                    ot[:],
                    v_s[:],
                    mybir.ActivationFunctionType.Identity,
                    bias=hb_sb[:, 0:1],
                )
                nc.sync.dma_start(out=o[s], in_=ot[:])

    nc.finalize()
    return nc


def _prepare_inputs(X, conv_w, conv_b, box_w, box_b, cls_w, cls_b):
    X = np.ascontiguousarray(np.asarray(X, dtype=np.float32))
    conv_w = np.asarray(conv_w, dtype=np.float32)

    # conv lhsT: [k=cin%128, idx*128+m], idx=(ci*9+ky*3+kx)*2+g, m=cout%128
    w6 = conv_w.reshape(2, 128, 2, 128, 3, 3)  # [g, m, ci, k, ky, kx]
    wc_host = np.ascontiguousarray(
        w6.transpose(3, 2, 4, 5, 0, 1).reshape(128, 36 * 128)
    )  # [k, (ci, ky, kx, g, m)] -> need idx-major: (ci,ky,kx,g) major then m

    # head lhsT: [k=cout%128, g*21+o], scaled by dv_c
    hw = np.concatenate(
        [np.asarray(box_w, np.float32)[:, :, 0, 0], np.asarray(cls_w, np.float32)[:, :, 0, 0]],
        axis=0,
    )  # [21, 256]
    wh_host = np.ascontiguousarray(
        (DV_C * hw).reshape(NO, 2, 128).transpose(2, 1, 0).reshape(128, 2 * NO)
    )

    cb_host = np.ascontiguousarray(np.asarray(conv_b, np.float32).reshape(2, 128).T)
    hb_host = np.ascontiguousarray(
        np.concatenate([np.asarray(box_b, np.float32), np.asarray(cls_b, np.float32)]).reshape(NO, 1)
    )

    xpad = np.zeros((TS, B, C, H + 2, W + 2), dtype=np.float32)
    xpad[:, :, :, 1 : H + 1, 1 : W + 1] = X

    in_maps = []
    for core in range(8):
        bi, hi = divmod(core, 2)
        slab = xpad[:, bi, :, hi * HH : hi * HH + PH, :]  # [16, 256, 26, 50]
        xp_host = np.ascontiguousarray(slab.reshape(TS, 2, 128, PH * PW))
        in_maps.append(
            {
                "Xp": xp_host,
                "Wc": wc_host,
                "Wh": wh_host,
                "CB": cb_host,
                "HB": hb_host,
            }
        )
    return in_maps


def _run(inputs, trace=False):
    global _CACHED
    if _CACHED is None:
        _CACHED = _build()
    nc = _CACHED
    in_maps = _prepare_inputs(**inputs)
    res = run_bass_kernel_spmd(nc, in_maps, core_ids=list(range(8)), trace=trace)

    boxes = np.empty((TS, B, OB, H, W), dtype=np.float32)
    classes = np.empty((TS, B, OC, H, W), dtype=np.float32)
    for core in range(8):
        bi, hi = divmod(core, 2)
        oc = res.results[core]["O"].reshape(TS, NO, HH, W)
        boxes[:, bi, :, hi * HH : (hi + 1) * HH, :] = oc[:, :OB]
        classes[:, bi, :, hi * HH : (hi + 1) * HH, :] = oc[:, OB:]
    return (boxes, classes), res


def kernel(**inputs):
    out, _ = _run(inputs, trace=False)
    return out


# revision 7
# speedup vs baseline: 595.4418x; 595.4418x over previous
"""Trainium2 Bass kernel for nn_Decoder_14894946582933.

Key fusion: conv3x3 (256->256), the 1x1 heads (256->21), the dv_c LI factor,
and conv_b are ALL linear maps on channels, so they fold into a single
3x3 conv with 21 output channels computed on host:
    W2[o,ci,ky,kx] = sum_c dv_c*head_w[o,c] * conv_w[c,ci,ky,kx]
    bias2[o]       = sum_c dv_c*head_w[o,c] * conv_b[c]
The kernel never materializes the 256-channel conv output.

Per timestep s, per core:
    Z[s]  = conv3x3_W2(X[s]) + bias2       (PE: 9 offsets x 2 cin-chunks, M=21)
    w_s   = alpha * w_{s-1} + Z[s]         (DVE scalar_tensor_tensor; w := dv_c*i)
    v_s   = beta  * v_{s-1} + w_{s-1}      (DVE; LI membrane voltage, emitted)
    out[s] = v_s + head_b                  (ACT bias add)

(The LI scan commutes with the channel contraction because both are linear.)

Sharding: 8 cores = 4 batch x 2 H-halves (halo of 1 row via host padding).
No collectives. Matmuls run in float32r (TF32-class, 1 cycle/row).
"""

import numpy as np

import concourse.mybir as mybir
from concourse import bacc
from concourse.tile import TileContext
from concourse.bass_utils import run_bass_kernel_spmd

# Problem shapes (hardcoded per contract)
TS = 16          # timesteps
B = 4            # batch
C = 256          # channels
H = W = 48       # spatial
OB, OC = 12, 9   # box / class output channels
NO = OB + OC     # 21 head outputs
HH = 24          # H rows per core (H/2)
PH, PW = HH + 2, W + 2   # padded slab: 26 x 50
ROWS = 8         # output rows per spatial chunk
NCH = HH // ROWS          # 3 spatial chunks
NF = ROWS * W             # 384 matmul free size
PIX = HH * W              # 1152 pixels per core

# norse LIParameters
DT_ = 1e-3
DV_C = DT_ * 100.0   # 0.1
DI_C = DT_ * 200.0   # 0.2
ALPHA = 1.0 - DI_C   # 0.8
BETA = 1.0 - DV_C    # 0.9

F32 = mybir.dt.float32
F32R = mybir.dt.float32r

_CACHED = None


def _build(repeat=1):
    nc = bacc.Bacc(None, target_bir_lowering=False)

    xp = nc.dram_tensor("Xp", [TS, 2, 128, PH * PW], F32R, kind="ExternalInput")
    w2 = nc.dram_tensor("W2", [128, 18 * NO], F32R, kind="ExternalInput")
    b2 = nc.dram_tensor("B2", [NO, 1], F32, kind="ExternalInput")
    hb = nc.dram_tensor("HB", [NO, 1], F32, kind="ExternalInput")
    o = nc.dram_tensor("O", [TS, NO, PIX], F32, kind="ExternalOutput")

    with TileContext(nc) as tc:
        with (
            tc.tile_pool(name="wpool", bufs=1) as wpool,
            tc.tile_pool(name="xpool", bufs=5) as xpool,
            tc.tile_pool(name="zpool", bufs=4) as zpool,
            tc.tile_pool(name="spool", bufs=2) as spool,
            tc.tile_pool(name="opool", bufs=2) as opool,
            tc.tile_pool(name="zpsum", bufs=6, space="PSUM") as zpsum,
        ):
            w2_sb = wpool.tile([128, 18 * NO], F32R)
            b2_sb = wpool.tile([NO, 1], F32)
            hb_sb = wpool.tile([NO, 1], F32)
            nc.sync.dma_start(out=w2_sb[:], in_=w2[:])
            nc.sync.dma_start(out=b2_sb[:], in_=b2[:])
            nc.sync.dma_start(out=hb_sb[:], in_=hb[:])

            for _rep in range(repeat):
                # LI state (v, w); start at zero
                v_prev = spool.tile([NO, PIX], F32, tag="v")
                w_prev = spool.tile([NO, PIX], F32, tag="w")
                nc.vector.memset(v_prev[:], 0.0)
                nc.vector.memset(w_prev[:], 0.0)

                for s in range(TS):
                    xts = []
                    for ci in range(2):
                        xt = xpool.tile([128, PH * PW], F32R, tag=f"x{ci}")
                        nc.sync.dma_start(out=xt[:], in_=xp[s, ci])
                        xts.append(xt.rearrange("p (h w) -> p h w", h=PH))

                    # fused conv+head: Z[s] = W2 * X[s] (+ bias2 on evac)
                    zsbs = []
                    for c in range(NCH):
                        zp = zpsum.tile([NO, NF], F32, tag="zp")
                        n_mm = 0
                        for ci in range(2):
                            for ky in range(3):
                                for kx in range(3):
                                    j = ci * 9 + ky * 3 + kx
                                    rhs = xts[ci][
                                        :,
                                        c * ROWS + ky : c * ROWS + ky + ROWS,
                                        kx : kx + W,
                                    ]
                                    nc.tensor.matmul(
                                        zp[:],
                                        w2_sb[:, j * NO : (j + 1) * NO],
                                        rhs,
                                        start=(n_mm == 0),
                                        stop=(n_mm == 17),
                                    )
                                    n_mm += 1
                        zsb = zpool.tile([NO, NF], F32, tag="z")
                        nc.scalar.activation(
                            zsb[:],
                            zp[:],
                            mybir.ActivationFunctionType.Identity,
                            bias=b2_sb[:, 0:1],
                        )
                        zsbs.append(zsb)

                    # LI scan step (order matters: v uses w_{s-1})
                    v_s = spool.tile([NO, PIX], F32, tag="v")
                    nc.vector.scalar_tensor_tensor(
                        out=v_s[:],
                        in0=v_prev[:],
                        scalar=BETA,
                        in1=w_prev[:],
                        op0=mybir.AluOpType.mult,
                        op1=mybir.AluOpType.add,
                    )
                    w_s = spool.tile([NO, PIX], F32, tag="w")
                    for c in range(NCH):
                        nc.vector.scalar_tensor_tensor(
                            out=w_s[:, c * NF : (c + 1) * NF],
                            in0=w_prev[:, c * NF : (c + 1) * NF],
                            scalar=ALPHA,
                            in1=zsbs[c][:],
                            op0=mybir.AluOpType.mult,
                            op1=mybir.AluOpType.add,
                        )
                    v_prev, w_prev = v_s, w_s

                    # output = v_s + head bias
                    ot = opool.tile([NO, PIX], F32, tag="o")
                    nc.scalar.activation(
                        ot[:],
                        v_s[:],
                        mybir.ActivationFunctionType.Identity,
                        bias=hb_sb[:, 0:1],
                    )
                    nc.sync.dma_start(out=o[s], in_=ot[:])

    nc.finalize()
    return nc


def _prepare_inputs(X, conv_w, conv_b, box_w, box_b, cls_w, cls_b):
    X = np.ascontiguousarray(np.asarray(X, dtype=np.float32))
    conv_w = np.asarray(conv_w, dtype=np.float32)
    conv_b = np.asarray(conv_b, dtype=np.float32)

    hw = np.concatenate(
        [np.asarray(box_w, np.float32)[:, :, 0, 0], np.asarray(cls_w, np.float32)[:, :, 0, 0]],
        axis=0,
    )  # [21, 256]
    hw_s = (DV_C * hw).astype(np.float64)

    # fused weights: W2[o, cin, ky, kx] = sum_c hw_s[o,c] * conv_w[c, cin, ky, kx]
    w2 = np.einsum("oc,cikl->oikl", hw_s, conv_w.astype(np.float64))
    # lhsT layout [k, j*21+o], j = ci*9 + ky*3 + kx, k = cin % 128
    w2_host = np.ascontiguousarray(
        w2.reshape(NO, 2, 128, 3, 3).transpose(2, 1, 3, 4, 0).reshape(128, 18 * NO)
    ).astype(np.float32)

    b2_host = np.ascontiguousarray(
        (hw_s @ conv_b.astype(np.float64)).astype(np.float32).reshape(NO, 1)
    )
    hb_host = np.ascontiguousarray(
        np.concatenate([np.asarray(box_b, np.float32), np.asarray(cls_b, np.float32)]).reshape(NO, 1)
    )

    xpad = np.zeros((TS, B, C, H + 2, W + 2), dtype=np.float32)
    xpad[:, :, :, 1 : H + 1, 1 : W + 1] = X

    in_maps = []
    for core in range(8):
        bi, hi = divmod(core, 2)
        slab = xpad[:, bi, :, hi * HH : hi * HH + PH, :]  # [16, 256, 26, 50]
        xp_host = np.ascontiguousarray(slab.reshape(TS, 2, 128, PH * PW))
        in_maps.append(
            {"Xp": xp_host, "W2": w2_host, "B2": b2_host, "HB": hb_host}
        )
    return in_maps


def _run(inputs, trace=False):
    global _CACHED
    if _CACHED is None:
        _CACHED = _build()
    nc = _CACHED
    in_maps = _prepare_inputs(**inputs)
    res = run_bass_kernel_spmd(nc, in_maps, core_ids=list(range(8)), trace=trace)

    boxes = np.empty((TS, B, OB, H, W), dtype=np.float32)
    classes = np.empty((TS, B, OC, H, W), dtype=np.float32)
    for core in range(8):
        bi, hi = divmod(core, 2)
        oc = res.results[core]["O"].reshape(TS, NO, HH, W)
        boxes[:, bi, :, hi * HH : (hi + 1) * HH, :] = oc[:, :OB]
        classes[:, bi, :, hi * HH : (hi + 1) * HH, :] = oc[:, OB:]
    return (boxes, classes), res


def kernel(**inputs):
    out, _ = _run(inputs, trace=False)
    return out
